# revision 30
# baseline (speedup 1.0000x reference)
"""GLIFR RNN (nn_BNNFC) Trainium2 Bass kernel — 8-core batch-data-parallel.

Strategy
--------
- Batch (64) sharded 8 ways -> 8 batch elements per core; weights replicated.
- The 20-step synaptic delay means the lateral matmul input firing(t-20) is
  known a whole block of 20 steps in advance, so lateral/input/readout
  matmuls run as batched [*, (t,b)] matmuls per 20-step block on TensorE.
- Only the elementwise state recurrence (asc currents, voltage, sigmoid) is
  truly sequential: 8 VectorE ops + 1 ScalarE sigmoid + 3 GPSIMD ops per
  step on [128, (h_outer=8, b=8)] tiles (H=1024 split as h = j*128 + p),
  refactored so only mul+add+sigmoid sit on the step-to-step chain.
- All rate constants are folded host-side:
    sg = sigmoid(trans_k_m); c1 = R*sg; c2 = 1-sg
    W_in' = W_in*c1, W_lat' = W_lat*c1 (column-scaled)
    A := c1*asc  =>  A(t) = (p*u+q)*A(t-1) + s'*u,  p=r*dka, q=1-dka,
    s' = c1*dka*amp;  vs := volt-thresh:
    vs(t) = syn'(t) + A1(t)+A2(t) + c2*vs(t-1),  syn' = c1*syn - sg*thresh
    firing(t) = sigmoid(vs(t))
"""

import os
import numpy as np
import ml_dtypes

import concourse.bacc as bacc
import concourse.tile as tile
import concourse.mybir as mybir
from concourse.bass_utils import run_bass_kernel_spmd

# problem constants
B, T, IN, HID, OUT = 64, 200, 512, 1024, 512
DELAY, NA = 20, 2
R_MEM = 0.1
N_CORES = 8
BC = B // N_CORES            # 8 batch per core
J = HID // 128               # 8 hidden chunks
KCI = IN // 128              # 4 input contraction chunks
OC = OUT // 128              # 4 output chunks
NBLK = T // DELAY            # 10 blocks of 20 steps
TB = DELAY                   # steps per block

MM_DT_S = os.environ.get("GLIFR_MM_DT", "bf16")   # matmul operand dtype
EW_DT_S = os.environ.get("GLIFR_EW_DT", "bf16")   # elementwise state dtype
ABLATE = os.environ.get("GLIFR_ABLATE", "")       # dev-only timing bisect

_DT = {"f32": mybir.dt.float32, "bf16": mybir.dt.bfloat16}
_NP = {"f32": np.float32, "bf16": ml_dtypes.bfloat16}

_CACHE = {}


def _build(mm_s, ew_s):
    mm = _DT[mm_s]
    ew = _DT[ew_s]
    f32 = mybir.dt.float32
    Act = mybir.ActivationFunctionType

    nc = bacc.Bacc("TRN2", target_bir_lowering=False, debug=False,
                   num_devices=N_CORES)

    # ---- DRAM parameters (per-core) ----
    d_xT = nc.dram_tensor("xT", [KCI, 128, T, BC], mm, kind="ExternalInput")
    d_win = nc.dram_tensor("w_in", [KCI, 128, HID], mm, kind="ExternalInput")
    d_wlat = nc.dram_tensor("w_lat", [J, 128, HID], mm, kind="ExternalInput")
    d_wout = nc.dram_tensor("w_out", [J, 128, OUT], mm, kind="ExternalInput")
    d_cP = nc.dram_tensor("cP", [128, NA, J, BC], ew, kind="ExternalInput")
    d_cQ = nc.dram_tensor("cQ", [128, NA, J, BC], ew, kind="ExternalInput")
    d_cS = nc.dram_tensor("cS", [128, NA, J, BC], ew, kind="ExternalInput")
    d_cC2 = nc.dram_tensor("cC2", [128, J, BC], ew, kind="ExternalInput")
    d_d10 = nc.dram_tensor("d1_0", [128, J, BC], ew, kind="ExternalInput")
    d_biasx = nc.dram_tensor("bias_x", [128, J], f32, kind="ExternalInput")
    d_bout = nc.dram_tensor("b_outT", [128, OC], f32, kind="ExternalInput")
    d_out = nc.dram_tensor("outT", [OC, 128, T, BC], f32, kind="ExternalOutput")

    HB = TB // 2   # lateral half-block = 10 steps

    with tile.TileContext(nc) as tc:
        with (
            tc.tile_pool(name="weights", bufs=1) as wpool,
            tc.tile_pool(name="state", bufs=1) as spool,
            tc.tile_pool(name="ew", bufs=2) as epool,
            tc.tile_pool(name="synp", bufs=2) as synpool,
            tc.tile_pool(name="ost", bufs=2) as opool,
            tc.tile_pool(name="ps_xp", bufs=2, space="PSUM") as psxp,
            tc.tile_pool(name="ps_lat", bufs=1, space="PSUM") as pslat,
            tc.tile_pool(name="ps_ro", bufs=1, space="PSUM") as psro,
        ):
            # ---- persistent tiles ----
            t_win = wpool.tile([128, KCI, HID], mm, tag="win")
            t_wlat = wpool.tile([128, J, HID], mm, tag="wlat")
            t_wout = wpool.tile([128, J, OUT], mm, tag="wout")
            t_cP = wpool.tile([128, NA, J, BC], ew, tag="cP")
            t_cQ = wpool.tile([128, NA, J, BC], ew, tag="cQ")
            t_cS = wpool.tile([128, NA, J, BC], ew, tag="cS")
            t_cC2 = wpool.tile([128, J, BC], ew, tag="cC2")
            t_biasx = wpool.tile([128, J], f32, tag="biasx")
            t_bout = wpool.tile([128, OC], f32, tag="bout")
            t_xT = wpool.tile([128, KCI, T, BC], mm, tag="xT")
            # xsyn = c1*x@W_in - sg*thresh, whole sequence. Always bf16:
            # halves SBUF and its quantization (~4e-4 of a ~0.1-scale value)
            # is far below the matmul dtype error in any configuration.
            t_xsyn = wpool.tile([128, J, T, BC], mybir.dt.bfloat16, tag="xsyn")

            # F_buf slot s holds firing(s-1); slot 0 = zeros
            t_F = spool.tile([128, J, T + 1, BC], mm, tag="F")
            t_A = spool.tile([128, NA, J, BC], ew, tag="A")
            t_Aq = spool.tile([128, NA, J, BC], ew, tag="Aq")
            t_Bst = spool.tile([128, NA, J, BC], ew, tag="Bst")
            t_vs = [spool.tile([128, J, BC], ew, tag=f"vs{i}", name=f"vs{i}")
                    for i in range(2)]
            t_D = [spool.tile([128, J, BC], ew, tag=f"D{i}", name=f"D{i}")
                    for i in range(2)]

            # ---- input DMAs ----
            # small, latency-critical transfers first
            nc.sync.dma_start(out=t_Bst[:], in_=d_cS.ap())
            nc.sync.dma_start(out=t_biasx[:], in_=d_biasx.ap())
            nc.sync.dma_start(out=t_cP[:], in_=d_cP.ap())
            nc.sync.dma_start(out=t_cQ[:], in_=d_cQ.ap())
            nc.sync.dma_start(out=t_cS[:], in_=d_cS.ap())
            nc.sync.dma_start(out=t_cC2[:], in_=d_cC2.ap())
            # x head (first 20 steps) + W_in unblock the first xproj
            # chunk; bulk transfers follow.
            nc.sync.dma_start(out=t_xT[:, :, 0:20, :],
                              in_=d_xT.ap()[:, :, 0:20, :]
                                  .rearrange("k p t b -> p k t b"))
            nc.sync.dma_start(out=t_win[:],
                              in_=d_win.ap().rearrange("k p h -> p k h"))
            nc.sync.dma_start(out=t_xT[:, :, 20:T, :],
                              in_=d_xT.ap()[:, :, 20:T, :]
                                  .rearrange("k p t b -> p k t b"))
            nc.sync.dma_start(out=t_wlat[:],
                              in_=d_wlat.ap().rearrange("k p h -> p k h"))
            nc.sync.dma_start(out=t_wout[:],
                              in_=d_wout.ap().rearrange("k p o -> p k o"))
            # D(-1) = d1(-1) + syn'(0) assembled on device: host sends
            # d1(-1) = -c2*thresh; add syn'(0) once xsyn chunk 0 exists.
            t_d1init = wpool.tile([128, J, BC], ew, tag="d1init")
            nc.sync.dma_start(out=t_d1init[:], in_=d_d10.ap())
            nc.sync.dma_start(out=t_bout[:], in_=d_bout.ap())

            # ---- state init ----
            nc.vector.memset(t_Aq[:], 0.0)
            nc.vector.memset(t_F[:, :, 0, :], 0.0)

            # upfront xproj time-chunks; small first chunk so block 0's
            # EW starts as early as possible (N = len*BC <= 512)
            XCHUNKS = [(0, 20), (20, 50), (70, 50), (120, 50), (170, 30)]

            def emit_xproj_chunk(tci, j):
                """xsyn[:, j, tc] = c1*x@W_in - sg*thresh for one time chunk."""
                lo, ln = XCHUNKS[tci]
                if "no_mm" in ABLATE:
                    if j == 0:
                        nc.gpsimd.memset(t_xsyn[:, :, lo:lo + ln, :], 0.0)
                    return
                ps = psxp.tile([128, 64, BC], f32, tag="xp")
                for kc in range(KCI):
                    nc.tensor.matmul(
                        out=ps[:, 0:ln, :],
                        lhsT=t_win[:, kc, j * 128:(j + 1) * 128],
                        rhs=t_xT[:, kc, lo:lo + ln, :],
                        start=(kc == 0), stop=(kc == KCI - 1))
                nc.scalar.activation(
                    out=t_xsyn[:, j, lo:lo + ln, :], in_=ps[:, 0:ln, :],
                    func=Act.Identity, bias=t_biasx[:, j:j + 1], scale=1.0)

            def emit_lat_group(k, ps, j, h):
                """lateral for block k, chunk j, 10-step half h: one psum
                accumulation group (start..stop). Half 0 only needs the first
                half of block k-1's firing -> closes during block k-1's EW."""
                if "no_mm" in ABLATE or "no_lat" in ABLATE:
                    return
                s0 = (k - 1) * TB + 1 + h * HB
                for kc in range(J):
                    nc.tensor.matmul(
                        out=ps[:, j, h * HB * BC:(h + 1) * HB * BC].rearrange(
                            "p (t b) -> p t b", t=HB),
                        lhsT=t_wlat[:, kc, j * 128:(j + 1) * 128],
                        rhs=t_F[:, kc, s0:s0 + HB, :],
                        start=(kc == 0), stop=(kc == J - 1))

            def emit_syn_half(k, ps, syn, j, h):
                """syn_sb[j, half] = lat_psum + xsyn  (ACT copy + GPSIMD add;
                GPSIMD cannot read PSUM)."""
                lsb = epool.tile([128, HB, BC], ew, tag="lsb")
                if "no_mm" in ABLATE or "no_lat" in ABLATE:
                    nc.gpsimd.memset(lsb[:], 0.0)
                else:
                    nc.scalar.activation(
                        out=lsb[:],
                        in_=ps[:, j, h * HB * BC:(h + 1) * HB * BC].rearrange(
                            "p (t b) -> p t b", t=HB),
                        func=Act.Identity, scale=1.0)
                nc.gpsimd.tensor_add(
                    out=syn[:, j, h * HB:(h + 1) * HB, :], in0=lsb[:],
                    in1=t_xsyn[:, j, k * TB + h * HB:k * TB + (h + 1) * HB, :])

            def emit_ro(k, deferred, deferred2=None):
                """readout matmuls + copies + DMA for block k. With
                deferred2, matmuls split into t-halves: half 0 goes to
                deferred2 (consumable during EW(k) second half)."""
                if "no_mm" in ABLATE or "no_ro" in ABLATE:
                    return
                ps = psro.tile([128, OC, 256], f32, tag="ro")
                s0 = k * TB + 1
                HB2 = TB // 2
                if deferred2 is not None:
                    for oc in range(OC):
                        for h in range(2):
                            tgt = deferred2 if h == 0 else deferred
                            for kc in range(J):
                                tgt.append(lambda oc=oc, kc=kc, h=h, ps=ps:
                                    nc.tensor.matmul(
                                        out=ps[:, oc, h * HB2 * BC:(h + 1) * HB2 * BC]
                                            .rearrange("p (t b) -> p t b", t=HB2),
                                        lhsT=t_wout[:, kc, oc * 128:(oc + 1) * 128],
                                        rhs=t_F[:, kc, s0 + h * HB2:s0 + (h + 1) * HB2, :],
                                        start=(kc == 0), stop=(kc == J - 1)))
                else:
                    for oc in range(OC):
                        for kc in range(J):
                            deferred.append(lambda oc=oc, kc=kc, ps=ps: nc.tensor.matmul(
                                out=ps[:, oc, 0:TB * BC].rearrange(
                                    "p (t b) -> p t b", t=TB),
                                lhsT=t_wout[:, kc, oc * 128:(oc + 1) * 128],
                                rhs=t_F[:, kc, s0:s0 + TB, :],
                                start=(kc == 0), stop=(kc == J - 1)))

                def emit_store(oc, ps=ps):
                    ot = opool.tile([128, TB, BC], f32, tag="ost")
                    nc.scalar.activation(
                        out=ot[:],
                        in_=ps[:, oc, 0:TB * BC].rearrange(
                            "p (t b) -> p t b", t=TB),
                        func=Act.Identity,
                        bias=t_bout[:, oc:oc + 1], scale=1.0)
                    nc.sync.dma_start(
                        out=d_out.ap()[oc, :, k * TB:(k + 1) * TB, :],
                        in_=ot[:])
                for oc in range(OC):
                    deferred.append(lambda oc=oc: emit_store(oc))

            def emit_ew_step(t, syn, syn_funcs):
                """B-form recurrence step; reads F slot t, writes slot t+1.

                Critical path after sigma(t-1): mb2 -> msum -> vs -> sigma(t).
                Everything else overlaps the ScalarE sigmoid round-trip; the
                d1 update runs on the GPSIMD engine. vs and d1 are
                double-buffered (t%2) to break cross-engine WAR stalls.
                """
                if "no_ew" in ABLATE:
                    return
                cur, prv = t % 2, (t + 1) % 2
                u2 = t_F[:, :, t, :].unsqueeze(1) \
                    .broadcast_to([128, NA, J, BC])
                # critical: vs(t) = u(t)*(B0+B1)(t-1) + D(t-1)
                mb2 = epool.tile([128, NA, J, BC], ew, tag="mb2")
                msum = epool.tile([128, J, BC], ew, tag="msum")
                with tc.high_priority(offset=40):
                    nc.vector.tensor_mul(out=mb2[:], in0=u2, in1=t_Bst[:])
                    nc.vector.tensor_add(out=msum[:], in0=mb2[:, 0],
                                         in1=mb2[:, 1])
                    nc.vector.tensor_add(out=t_vs[cur][:], in0=msum[:],
                                         in1=t_D[prv][:])
                    if "no_sigma" not in ABLATE:
                        nc.scalar.activation(out=t_F[:, :, t + 1, :],
                                             in_=t_vs[cur][:], func=Act.Sigmoid)
                # state updates (overlap sigma): A(t) = Aq(t-1) + mb2
                nc.vector.tensor_add(out=t_A[:], in0=t_Aq[:], in1=mb2[:])
                nc.vector.tensor_mul(out=t_Aq[:], in0=t_A[:], in1=t_cQ[:])
                qa = epool.tile([128, J, BC], ew, tag="qa")
                nc.vector.tensor_add(out=qa[:], in0=t_Aq[:, 0], in1=t_Aq[:, 1])
                # B(t) = p*A(t) + s'   (on the step loop -> keep on DVE)
                bp = epool.tile([128, NA, J, BC], ew, tag="bp")
                nc.vector.tensor_mul(out=bp[:], in0=t_A[:], in1=t_cP[:])
                nc.vector.tensor_add(out=t_Bst[:], in0=bp[:], in1=t_cS[:])
                # D(t) = c2*vs(t) + QA(t) + syn'(t+1): on GPSIMD, with a
                # full step of slack before vs(t+1) consumes it.
                cv = epool.tile([128, J, BC], ew, tag="cv")
                nc.gpsimd.tensor_mul(out=cv[:], in0=t_vs[cur][:], in1=t_cC2[:])
                d1 = epool.tile([128, J, BC], ew, tag="d1w")
                nc.gpsimd.tensor_add(out=d1[:], in0=cv[:], in1=qa[:])
                if t + 1 < T:
                    nxt = syn_funcs[(t + 1) // TB]
                    nc.gpsimd.tensor_add(out=t_D[cur][:], in0=d1[:],
                                         in1=nxt(t + 1))

            # ---------- main schedule ----------
            # Upfront input projection, first time-chunk first so block 0's
            # EW can start; the rest overlaps early blocks.
            xp_todo = []
            for tci in range(len(XCHUNKS)):
                for j in range(J):
                    if tci == 0:
                        emit_xproj_chunk(tci, j)
                    else:
                        xp_todo.append(lambda tci=tci, j=j:
                                       emit_xproj_chunk(tci, j))

            nc.gpsimd.tensor_add(out=t_D[1][:], in0=t_d1init[:],
                                 in1=t_xsyn[:, :, 0, :])

            def xsyn_slice(k):
                def f(t):
                    return t_xsyn[:, :, t, :]
                return f

            def synsb_slice(syn):
                def f(t):
                    return syn[:, :, t % TB, :]
                return f


            syn_funcs = {0: xsyn_slice(0)}   # block 0 reads xsyn directly
            ps_next = None
            syn_next = None
            for k in range(NBLK):
                # defA: popped during EW steps 0..8: remaining upfront
                #   xproj chunks and block k-1's readout.
                # defB: popped during EW steps 10..18: block k+1 lateral
                #   half-0 groups + their syn assembly (consume this block's
                #   first-half firing as it appears).
                defA, defB = [], []
                # one xproj chunk (8 j-emissions) per block: dumping them all
                # into block 0 queues ~21us of PE work ahead of block 1's
                # lateral and delays its syn assembly
                defA.extend(xp_todo[:J])
                xp_todo = xp_todo[J:]
                if k >= 1:
                    emit_ro(k - 1, defA)
                if k + 1 < NBLK:
                    ps_next = pslat.tile([128, J, 256], f32, tag="lat")
                    syn_next = synpool.tile([128, J, TB, BC], ew, tag="syn_sb")
                    syn_funcs[k + 1] = synsb_slice(syn_next)
                    for j in range(J):
                        defB.append(lambda j=j, ps=ps_next:
                                    emit_lat_group(k + 1, ps, j, 0))
                        defB.append(lambda j=j, ps=ps_next, sy=syn_next:
                                    emit_syn_half(k + 1, ps, sy, j, 0))
                if k == NBLK - 1:
                    ro_tail = []
                    emit_ro(NBLK - 1, ro_tail, deferred2=defB)

                perA = max(1, (len(defA) + 8) // 9)
                perB = max(1, (len(defB) + 8) // 9)
                for li, t in enumerate(range(k * TB, (k + 1) * TB)):
                    emit_ew_step(t, syn_funcs[k], syn_funcs)
                    pend, per = (defA, perA) if li < 10 else (defB, perB)
                    for _ in range(per):
                        if pend:
                            pend.pop(0)()
                for fn in defA + defB:
                    fn()

                # post-EW(k): block k+1 lateral half-1 (waits on this block's
                # last sigmoid, runs while EW(k+1) steps 0..9 execute).
                if k + 1 < NBLK:
                    for j in range(J):
                        emit_lat_group(k + 1, ps_next, j, 1)
                        emit_syn_half(k + 1, ps_next, syn_next, j, 1)

            # final readout tail (half 1 + stores; half 0 ran in EW(9))
            for fn in ro_tail:
                fn()

    nc.compile()
    return nc


def _sigmoid(x):
    return 1.0 / (1.0 + np.exp(-x))


def _prep(inputs, mm_s, ew_s):
    mmn = _NP[mm_s]
    ewn = _NP[ew_s]
    f32 = np.float32

    x = np.asarray(inputs["x"], f32)
    W_in = np.asarray(inputs["W_in"], f32)
    W_lat = np.asarray(inputs["W_lat"], f32)
    thresh = np.asarray(inputs["thresh"], f32)[0]
    trans_k_m = np.asarray(inputs["trans_k_m"], f32)[0]
    trans_asc_k = np.asarray(inputs["trans_asc_k"], f32)[:, 0, :]
    asc_amp = np.asarray(inputs["asc_amp"], f32)[:, 0, :]
    trans_asc_r = np.asarray(inputs["trans_asc_r"], f32)[:, 0, :]
    W_out = np.asarray(inputs["W_out"], f32)
    b_out = np.asarray(inputs["b_out"], f32)

    sg = _sigmoid(trans_k_m).astype(f32)
    c1 = (R_MEM * sg).astype(f32)
    c2 = (1.0 - sg).astype(f32)
    dka = _sigmoid(trans_asc_k).astype(f32)
    r_a = (1.0 - 2.0 * _sigmoid(trans_asc_r)).astype(f32)
    p_a = (r_a * dka).astype(f32)
    q_a = (1.0 - dka).astype(f32)
    s_a = (c1[None] * dka * asc_amp).astype(f32)
    bias_h = (-sg * thresh).astype(f32)

    w_in = (W_in * c1[None, :]).astype(mmn).reshape(KCI, 128, HID)
    w_lat = (W_lat * c1[None, :]).astype(mmn).reshape(J, 128, HID)
    w_out = np.ascontiguousarray(W_out.T).astype(mmn).reshape(J, 128, OUT)

    def hb(coef_ah):  # [NA,H] -> [128, NA, J, BC]
        a = coef_ah.reshape(NA, J, 128).transpose(2, 0, 1)
        return np.broadcast_to(a[..., None], (128, NA, J, BC)).astype(ewn).copy()

    def hb1(coef_h):  # [H] -> [128, J, BC]
        a = coef_h.reshape(J, 128).T
        return np.broadcast_to(a[..., None], (128, J, BC)).astype(ewn).copy()

    cP, cQ, cS = hb(p_a), hb(q_a), hb(s_a)
    cC2 = hb1(c2)
    d1_0 = hb1((-c2 * thresh).astype(f32))
    bias_x = np.ascontiguousarray(bias_h.reshape(J, 128).T).astype(f32)
    b_outT = np.ascontiguousarray(b_out.reshape(OC, 128).T).astype(f32)

    in_maps = []
    for c in range(N_CORES):
        xc = x[c * BC:(c + 1) * BC]                    # [8, 200, 512]
        xT = np.ascontiguousarray(xc.transpose(2, 1, 0)).astype(mmn) \
            .reshape(KCI, 128, T, BC)
        in_maps.append({
            "xT": xT, "w_in": w_in, "w_lat": w_lat, "w_out": w_out,
            "cP": cP, "cQ": cQ, "cS": cS, "cC2": cC2, "d1_0": d1_0,
            "bias_x": bias_x, "b_outT": b_outT,
        })
    return in_maps


def _get_nc():
    key = (MM_DT_S, EW_DT_S, ABLATE)
    if key not in _CACHE:
        _CACHE[key] = _build(MM_DT_S, EW_DT_S)
    return _CACHE[key]


def kernel(**inputs) -> np.ndarray:
    nc = _get_nc()
    in_maps = _prep(inputs, MM_DT_S, EW_DT_S)
    try:
        res = run_bass_kernel_spmd(nc, in_maps, list(range(N_CORES)))
    except Exception:
        # transient NRT device errors have been observed through the axon
        # tunnel; one retry normally succeeds
        import time as _time
        _time.sleep(2.0)
        res = run_bass_kernel_spmd(nc, in_maps, list(range(N_CORES)))
    out = np.empty((B, T, OUT), np.float32)
    for c in range(N_CORES):
        r = res.results[c]["outT"]                     # [OC, 128, T, BC]
        out[c * BC:(c + 1) * BC] = r.transpose(3, 2, 0, 1).reshape(BC, T, OUT)
    return out



# revision 31
# speedup vs baseline: 1.0149x; 1.0149x over previous
"""GLIFR RNN (nn_BNNFC) Trainium2 Bass kernel — 8-core batch-data-parallel.

Strategy
--------
- Batch (64) sharded 8 ways -> 8 batch elements per core; weights replicated.
- The 20-step synaptic delay means the lateral matmul input firing(t-20) is
  known a whole block of 20 steps in advance, so lateral/input/readout
  matmuls run as batched [*, (t,b)] matmuls per 20-step block on TensorE.
- Only the elementwise state recurrence (asc currents, voltage, sigmoid) is
  truly sequential: 8 VectorE ops + 1 ScalarE sigmoid + 3 GPSIMD ops per
  step on [128, (h_outer=8, b=8)] tiles (H=1024 split as h = j*128 + p),
  refactored so only mul+add+sigmoid sit on the step-to-step chain.
- All rate constants are folded host-side:
    sg = sigmoid(trans_k_m); c1 = R*sg; c2 = 1-sg
    W_in' = W_in*c1, W_lat' = W_lat*c1 (column-scaled)
    A := c1*asc  =>  A(t) = (p*u+q)*A(t-1) + s'*u,  p=r*dka, q=1-dka,
    s' = c1*dka*amp;  vs := volt-thresh:
    vs(t) = syn'(t) + A1(t)+A2(t) + c2*vs(t-1),  syn' = c1*syn - sg*thresh
    firing(t) = sigmoid(vs(t))
"""

import os
import numpy as np
import ml_dtypes

import concourse.bacc as bacc
import concourse.tile as tile
import concourse.mybir as mybir
from concourse.bass_utils import run_bass_kernel_spmd

# problem constants
B, T, IN, HID, OUT = 64, 200, 512, 1024, 512
DELAY, NA = 20, 2
R_MEM = 0.1
N_CORES = 8
BC = B // N_CORES            # 8 batch per core
J = HID // 128               # 8 hidden chunks
KCI = IN // 128              # 4 input contraction chunks
OC = OUT // 128              # 4 output chunks
NBLK = T // DELAY            # 10 blocks of 20 steps
TB = DELAY                   # steps per block

MM_DT_S = os.environ.get("GLIFR_MM_DT", "bf16")   # matmul operand dtype
EW_DT_S = os.environ.get("GLIFR_EW_DT", "bf16")   # elementwise state dtype
ABLATE = os.environ.get("GLIFR_ABLATE", "")       # dev-only timing bisect

_DT = {"f32": mybir.dt.float32, "bf16": mybir.dt.bfloat16}
_NP = {"f32": np.float32, "bf16": ml_dtypes.bfloat16}

_CACHE = {}


def _build(mm_s, ew_s):
    mm = _DT[mm_s]
    ew = _DT[ew_s]
    f32 = mybir.dt.float32
    Act = mybir.ActivationFunctionType

    nc = bacc.Bacc("TRN2", target_bir_lowering=False, debug=False,
                   num_devices=N_CORES)

    # ---- DRAM parameters (per-core) ----
    d_xT = nc.dram_tensor("xT", [KCI, 128, T, BC], mm, kind="ExternalInput")
    d_win = nc.dram_tensor("w_in", [KCI, 128, HID], mm, kind="ExternalInput")
    d_wlat = nc.dram_tensor("w_lat", [J, 128, HID], mm, kind="ExternalInput")
    d_wout = nc.dram_tensor("w_out", [J, 128, OUT], mm, kind="ExternalInput")
    d_cP = nc.dram_tensor("cP", [128, NA, J, BC], ew, kind="ExternalInput")
    d_cQ = nc.dram_tensor("cQ", [128, NA, J, BC], ew, kind="ExternalInput")
    d_cS = nc.dram_tensor("cS", [128, NA, J, BC], ew, kind="ExternalInput")
    d_cC2 = nc.dram_tensor("cC2", [128, J, BC], ew, kind="ExternalInput")
    d_d10 = nc.dram_tensor("d1_0", [128, J, BC], ew, kind="ExternalInput")
    d_biasx = nc.dram_tensor("bias_x", [128, J], f32, kind="ExternalInput")
    d_bout = nc.dram_tensor("b_outT", [128, OC], f32, kind="ExternalInput")
    d_out = nc.dram_tensor("outT", [OC, 128, T, BC], f32, kind="ExternalOutput")

    HB = TB // 2   # lateral half-block = 10 steps

    with tile.TileContext(nc) as tc:
        with (
            tc.tile_pool(name="weights", bufs=1) as wpool,
            tc.tile_pool(name="state", bufs=1) as spool,
            tc.tile_pool(name="ew", bufs=2) as epool,
            tc.tile_pool(name="synp", bufs=2) as synpool,
            tc.tile_pool(name="ost", bufs=2) as opool,
            tc.tile_pool(name="ps_xp", bufs=2, space="PSUM") as psxp,
            tc.tile_pool(name="ps_lat", bufs=1, space="PSUM") as pslat,
            tc.tile_pool(name="ps_ro", bufs=1, space="PSUM") as psro,
        ):
            # ---- persistent tiles ----
            t_win = wpool.tile([128, KCI, HID], mm, tag="win")
            t_wlat = wpool.tile([128, J, HID], mm, tag="wlat")
            t_wout = wpool.tile([128, J, OUT], mm, tag="wout")
            t_cP = wpool.tile([128, NA, J, BC], ew, tag="cP")
            t_cQ = wpool.tile([128, NA, J, BC], ew, tag="cQ")
            t_cS = wpool.tile([128, NA, J, BC], ew, tag="cS")
            t_cC2 = wpool.tile([128, J, BC], ew, tag="cC2")
            t_biasx = wpool.tile([128, J], f32, tag="biasx")
            t_bout = wpool.tile([128, OC], f32, tag="bout")
            t_xT = wpool.tile([128, KCI, T, BC], mm, tag="xT")
            # xsyn = c1*x@W_in - sg*thresh, whole sequence. Always bf16:
            # halves SBUF and its quantization (~4e-4 of a ~0.1-scale value)
            # is far below the matmul dtype error in any configuration.
            t_xsyn = wpool.tile([128, J, T, BC], mybir.dt.bfloat16, tag="xsyn")

            # F_buf slot s holds firing(s-1); slot 0 = zeros
            t_F = spool.tile([128, J, T + 1, BC], mm, tag="F")
            t_A = spool.tile([128, NA, J, BC], ew, tag="A")
            t_Aq = spool.tile([128, NA, J, BC], ew, tag="Aq")
            t_Bst = spool.tile([128, NA, J, BC], ew, tag="Bst")
            t_vs = [spool.tile([128, J, BC], ew, tag=f"vs{i}", name=f"vs{i}")
                    for i in range(2)]
            t_D = [spool.tile([128, J, BC], ew, tag=f"D{i}", name=f"D{i}")
                    for i in range(2)]

            # ---- input DMAs ----
            # small, latency-critical transfers first
            nc.gpsimd.dma_start(out=t_Bst[:], in_=d_cS.ap())
            nc.gpsimd.dma_start(out=t_biasx[:], in_=d_biasx.ap())
            nc.gpsimd.dma_start(out=t_cP[:], in_=d_cP.ap())
            nc.gpsimd.dma_start(out=t_cQ[:], in_=d_cQ.ap())
            nc.gpsimd.dma_start(out=t_cS[:], in_=d_cS.ap())
            nc.gpsimd.dma_start(out=t_cC2[:], in_=d_cC2.ap())
            # x head (first 20 steps) + W_in unblock the first xproj
            # chunk; bulk transfers follow.
            nc.sync.dma_start(out=t_xT[:, :, 0:20, :],
                              in_=d_xT.ap()[:, :, 0:20, :]
                                  .rearrange("k p t b -> p k t b"))
            nc.sync.dma_start(out=t_win[:],
                              in_=d_win.ap().rearrange("k p h -> p k h"))
            nc.sync.dma_start(out=t_xT[:, :, 20:T, :],
                              in_=d_xT.ap()[:, :, 20:T, :]
                                  .rearrange("k p t b -> p k t b"))
            nc.sync.dma_start(out=t_wlat[:],
                              in_=d_wlat.ap().rearrange("k p h -> p k h"))
            nc.sync.dma_start(out=t_wout[:],
                              in_=d_wout.ap().rearrange("k p o -> p k o"))
            # D(-1) = d1(-1) + syn'(0) assembled on device: host sends
            # d1(-1) = -c2*thresh; add syn'(0) once xsyn chunk 0 exists.
            t_d1init = wpool.tile([128, J, BC], ew, tag="d1init")
            nc.gpsimd.dma_start(out=t_d1init[:], in_=d_d10.ap())
            nc.gpsimd.dma_start(out=t_bout[:], in_=d_bout.ap())

            # ---- state init ----
            nc.vector.memset(t_Aq[:], 0.0)
            nc.vector.memset(t_F[:, :, 0, :], 0.0)

            # upfront xproj time-chunks; small first chunk so block 0's
            # EW starts as early as possible (N = len*BC <= 512)
            XCHUNKS = [(0, 4), (4, 16), (20, 50), (70, 50), (120, 50),
                       (170, 30)]

            def emit_xproj_chunk(tci, j):
                """xsyn[:, j, tc] = c1*x@W_in - sg*thresh for one time chunk."""
                lo, ln = XCHUNKS[tci]
                if "no_mm" in ABLATE:
                    if j == 0:
                        nc.gpsimd.memset(t_xsyn[:, :, lo:lo + ln, :], 0.0)
                    return
                ps = psxp.tile([128, 64, BC], f32, tag="xp")
                for kc in range(KCI):
                    nc.tensor.matmul(
                        out=ps[:, 0:ln, :],
                        lhsT=t_win[:, kc, j * 128:(j + 1) * 128],
                        rhs=t_xT[:, kc, lo:lo + ln, :],
                        start=(kc == 0), stop=(kc == KCI - 1))
                nc.scalar.activation(
                    out=t_xsyn[:, j, lo:lo + ln, :], in_=ps[:, 0:ln, :],
                    func=Act.Identity, bias=t_biasx[:, j:j + 1], scale=1.0)

            def emit_lat_group(k, ps, j, h):
                """lateral for block k, chunk j, 10-step half h: one psum
                accumulation group (start..stop). Half 0 only needs the first
                half of block k-1's firing -> closes during block k-1's EW."""
                if "no_mm" in ABLATE or "no_lat" in ABLATE:
                    return
                s0 = (k - 1) * TB + 1 + h * HB
                for kc in range(J):
                    nc.tensor.matmul(
                        out=ps[:, j, h * HB * BC:(h + 1) * HB * BC].rearrange(
                            "p (t b) -> p t b", t=HB),
                        lhsT=t_wlat[:, kc, j * 128:(j + 1) * 128],
                        rhs=t_F[:, kc, s0:s0 + HB, :],
                        start=(kc == 0), stop=(kc == J - 1))

            def emit_syn_half(k, ps, syn, j, h):
                """syn_sb[j, half] = lat_psum + xsyn  (ACT copy + GPSIMD add;
                GPSIMD cannot read PSUM)."""
                lsb = epool.tile([128, HB, BC], ew, tag="lsb")
                if "no_mm" in ABLATE or "no_lat" in ABLATE:
                    nc.gpsimd.memset(lsb[:], 0.0)
                else:
                    nc.scalar.activation(
                        out=lsb[:],
                        in_=ps[:, j, h * HB * BC:(h + 1) * HB * BC].rearrange(
                            "p (t b) -> p t b", t=HB),
                        func=Act.Identity, scale=1.0)
                nc.gpsimd.tensor_add(
                    out=syn[:, j, h * HB:(h + 1) * HB, :], in0=lsb[:],
                    in1=t_xsyn[:, j, k * TB + h * HB:k * TB + (h + 1) * HB, :])

            def emit_ro(k, deferred, deferred2=None):
                """readout matmuls + copies + DMA for block k. With
                deferred2, matmuls split into t-halves: half 0 goes to
                deferred2 (consumable during EW(k) second half)."""
                if "no_mm" in ABLATE or "no_ro" in ABLATE:
                    return
                ps = psro.tile([128, OC, 256], f32, tag="ro")
                s0 = k * TB + 1
                HB2 = TB // 2
                if deferred2 is not None:
                    for oc in range(OC):
                        for h in range(2):
                            tgt = deferred2 if h == 0 else deferred
                            for kc in range(J):
                                tgt.append(lambda oc=oc, kc=kc, h=h, ps=ps:
                                    nc.tensor.matmul(
                                        out=ps[:, oc, h * HB2 * BC:(h + 1) * HB2 * BC]
                                            .rearrange("p (t b) -> p t b", t=HB2),
                                        lhsT=t_wout[:, kc, oc * 128:(oc + 1) * 128],
                                        rhs=t_F[:, kc, s0 + h * HB2:s0 + (h + 1) * HB2, :],
                                        start=(kc == 0), stop=(kc == J - 1)))
                else:
                    for oc in range(OC):
                        for kc in range(J):
                            deferred.append(lambda oc=oc, kc=kc, ps=ps: nc.tensor.matmul(
                                out=ps[:, oc, 0:TB * BC].rearrange(
                                    "p (t b) -> p t b", t=TB),
                                lhsT=t_wout[:, kc, oc * 128:(oc + 1) * 128],
                                rhs=t_F[:, kc, s0:s0 + TB, :],
                                start=(kc == 0), stop=(kc == J - 1)))

                def emit_store(oc, ps=ps):
                    ot = opool.tile([128, TB, BC], f32, tag="ost")
                    nc.scalar.activation(
                        out=ot[:],
                        in_=ps[:, oc, 0:TB * BC].rearrange(
                            "p (t b) -> p t b", t=TB),
                        func=Act.Identity,
                        bias=t_bout[:, oc:oc + 1], scale=1.0)
                    nc.sync.dma_start(
                        out=d_out.ap()[oc, :, k * TB:(k + 1) * TB, :],
                        in_=ot[:])
                for oc in range(OC):
                    deferred.append(lambda oc=oc: emit_store(oc))

            def emit_ew_step(t, syn, syn_funcs):
                """B-form recurrence step; reads F slot t, writes slot t+1.

                Critical path after sigma(t-1): mb2 -> msum -> vs -> sigma(t).
                Everything else overlaps the ScalarE sigmoid round-trip; the
                d1 update runs on the GPSIMD engine. vs and d1 are
                double-buffered (t%2) to break cross-engine WAR stalls.
                """
                if "no_ew" in ABLATE:
                    return
                cur, prv = t % 2, (t + 1) % 2
                u2 = t_F[:, :, t, :].unsqueeze(1) \
                    .broadcast_to([128, NA, J, BC])
                # critical: vs(t) = u(t)*(B0+B1)(t-1) + D(t-1)
                mb2 = epool.tile([128, NA, J, BC], ew, tag="mb2")
                msum = epool.tile([128, J, BC], ew, tag="msum")
                with tc.high_priority(offset=40):
                    nc.vector.tensor_mul(out=mb2[:], in0=u2, in1=t_Bst[:])
                    nc.vector.tensor_add(out=msum[:], in0=mb2[:, 0],
                                         in1=mb2[:, 1])
                    nc.vector.tensor_add(out=t_vs[cur][:], in0=msum[:],
                                         in1=t_D[prv][:])
                    if "no_sigma" not in ABLATE:
                        nc.scalar.activation(out=t_F[:, :, t + 1, :],
                                             in_=t_vs[cur][:], func=Act.Sigmoid)
                # state updates (overlap sigma): A(t) = Aq(t-1) + mb2
                nc.vector.tensor_add(out=t_A[:], in0=t_Aq[:], in1=mb2[:])
                nc.vector.tensor_mul(out=t_Aq[:], in0=t_A[:], in1=t_cQ[:])
                qa = epool.tile([128, J, BC], ew, tag="qa")
                nc.vector.tensor_add(out=qa[:], in0=t_Aq[:, 0], in1=t_Aq[:, 1])
                # B(t) = p*A(t) + s'   (on the step loop -> keep on DVE)
                bp = epool.tile([128, NA, J, BC], ew, tag="bp")
                nc.vector.tensor_mul(out=bp[:], in0=t_A[:], in1=t_cP[:])
                nc.vector.tensor_add(out=t_Bst[:], in0=bp[:], in1=t_cS[:])
                # D(t) = c2*vs(t) + QA(t) + syn'(t+1): on GPSIMD, with a
                # full step of slack before vs(t+1) consumes it.
                cv = epool.tile([128, J, BC], ew, tag="cv")
                nc.gpsimd.tensor_mul(out=cv[:], in0=t_vs[cur][:], in1=t_cC2[:])
                d1 = epool.tile([128, J, BC], ew, tag="d1w")
                nc.gpsimd.tensor_add(out=d1[:], in0=cv[:], in1=qa[:])
                if t + 1 < T:
                    nxt = syn_funcs[(t + 1) // TB]
                    nc.gpsimd.tensor_add(out=t_D[cur][:], in0=d1[:],
                                         in1=nxt(t + 1))

            # ---------- main schedule ----------
            # Upfront input projection, first time-chunk first so block 0's
            # EW can start; the rest overlaps early blocks.
            xp_todo = []
            for tci in range(len(XCHUNKS)):
                for j in range(J):
                    if tci <= 1:
                        emit_xproj_chunk(tci, j)
                    else:
                        xp_todo.append(lambda tci=tci, j=j:
                                       emit_xproj_chunk(tci, j))

            nc.gpsimd.tensor_add(out=t_D[1][:], in0=t_d1init[:],
                                 in1=t_xsyn[:, :, 0, :])

            def xsyn_slice(k):
                def f(t):
                    return t_xsyn[:, :, t, :]
                return f

            def synsb_slice(syn):
                def f(t):
                    return syn[:, :, t % TB, :]
                return f


            syn_funcs = {0: xsyn_slice(0)}   # block 0 reads xsyn directly
            ps_next = None
            syn_next = None
            for k in range(NBLK):
                # defA: popped during EW steps 0..8: remaining upfront
                #   xproj chunks and block k-1's readout.
                # defB: popped during EW steps 10..18: block k+1 lateral
                #   half-0 groups + their syn assembly (consume this block's
                #   first-half firing as it appears).
                defA, defB = [], []
                # one xproj chunk (8 j-emissions) per block: dumping them all
                # into block 0 queues ~21us of PE work ahead of block 1's
                # lateral and delays its syn assembly
                defA.extend(xp_todo[:J])
                xp_todo = xp_todo[J:]
                if k >= 1:
                    emit_ro(k - 1, defA)
                if k + 1 < NBLK:
                    ps_next = pslat.tile([128, J, 256], f32, tag="lat")
                    syn_next = synpool.tile([128, J, TB, BC], ew, tag="syn_sb")
                    syn_funcs[k + 1] = synsb_slice(syn_next)
                    for j in range(J):
                        defB.append(lambda j=j, ps=ps_next:
                                    emit_lat_group(k + 1, ps, j, 0))
                        defB.append(lambda j=j, ps=ps_next, sy=syn_next:
                                    emit_syn_half(k + 1, ps, sy, j, 0))
                if k == NBLK - 1:
                    ro_tail = []
                    emit_ro(NBLK - 1, ro_tail, deferred2=defB)

                perA = max(1, (len(defA) + 8) // 9)
                perB = max(1, (len(defB) + 8) // 9)
                for li, t in enumerate(range(k * TB, (k + 1) * TB)):
                    emit_ew_step(t, syn_funcs[k], syn_funcs)
                    pend, per = (defA, perA) if li < 10 else (defB, perB)
                    for _ in range(per):
                        if pend:
                            pend.pop(0)()
                for fn in defA + defB:
                    fn()

                # post-EW(k): block k+1 lateral half-1 (waits on this block's
                # last sigmoid, runs while EW(k+1) steps 0..9 execute).
                if k + 1 < NBLK:
                    for j in range(J):
                        emit_lat_group(k + 1, ps_next, j, 1)
                        emit_syn_half(k + 1, ps_next, syn_next, j, 1)

            # final readout tail (half 1 + stores; half 0 ran in EW(9))
            for fn in ro_tail:
                fn()

    nc.compile()
    return nc


def _sigmoid(x):
    return 1.0 / (1.0 + np.exp(-x))


def _prep(inputs, mm_s, ew_s):
    mmn = _NP[mm_s]
    ewn = _NP[ew_s]
    f32 = np.float32

    x = np.asarray(inputs["x"], f32)
    W_in = np.asarray(inputs["W_in"], f32)
    W_lat = np.asarray(inputs["W_lat"], f32)
    thresh = np.asarray(inputs["thresh"], f32)[0]
    trans_k_m = np.asarray(inputs["trans_k_m"], f32)[0]
    trans_asc_k = np.asarray(inputs["trans_asc_k"], f32)[:, 0, :]
    asc_amp = np.asarray(inputs["asc_amp"], f32)[:, 0, :]
    trans_asc_r = np.asarray(inputs["trans_asc_r"], f32)[:, 0, :]
    W_out = np.asarray(inputs["W_out"], f32)
    b_out = np.asarray(inputs["b_out"], f32)

    sg = _sigmoid(trans_k_m).astype(f32)
    c1 = (R_MEM * sg).astype(f32)
    c2 = (1.0 - sg).astype(f32)
    dka = _sigmoid(trans_asc_k).astype(f32)
    r_a = (1.0 - 2.0 * _sigmoid(trans_asc_r)).astype(f32)
    p_a = (r_a * dka).astype(f32)
    q_a = (1.0 - dka).astype(f32)
    s_a = (c1[None] * dka * asc_amp).astype(f32)
    bias_h = (-sg * thresh).astype(f32)

    w_in = (W_in * c1[None, :]).astype(mmn).reshape(KCI, 128, HID)
    w_lat = (W_lat * c1[None, :]).astype(mmn).reshape(J, 128, HID)
    w_out = np.ascontiguousarray(W_out.T).astype(mmn).reshape(J, 128, OUT)

    def hb(coef_ah):  # [NA,H] -> [128, NA, J, BC]
        a = coef_ah.reshape(NA, J, 128).transpose(2, 0, 1)
        return np.broadcast_to(a[..., None], (128, NA, J, BC)).astype(ewn).copy()

    def hb1(coef_h):  # [H] -> [128, J, BC]
        a = coef_h.reshape(J, 128).T
        return np.broadcast_to(a[..., None], (128, J, BC)).astype(ewn).copy()

    cP, cQ, cS = hb(p_a), hb(q_a), hb(s_a)
    cC2 = hb1(c2)
    d1_0 = hb1((-c2 * thresh).astype(f32))
    bias_x = np.ascontiguousarray(bias_h.reshape(J, 128).T).astype(f32)
    b_outT = np.ascontiguousarray(b_out.reshape(OC, 128).T).astype(f32)

    in_maps = []
    for c in range(N_CORES):
        xc = x[c * BC:(c + 1) * BC]                    # [8, 200, 512]
        xT = np.ascontiguousarray(xc.transpose(2, 1, 0)).astype(mmn) \
            .reshape(KCI, 128, T, BC)
        in_maps.append({
            "xT": xT, "w_in": w_in, "w_lat": w_lat, "w_out": w_out,
            "cP": cP, "cQ": cQ, "cS": cS, "cC2": cC2, "d1_0": d1_0,
            "bias_x": bias_x, "b_outT": b_outT,
        })
    return in_maps


def _get_nc():
    key = (MM_DT_S, EW_DT_S, ABLATE)
    if key not in _CACHE:
        _CACHE[key] = _build(MM_DT_S, EW_DT_S)
    return _CACHE[key]


def kernel(**inputs) -> np.ndarray:
    nc = _get_nc()
    in_maps = _prep(inputs, MM_DT_S, EW_DT_S)
    try:
        res = run_bass_kernel_spmd(nc, in_maps, list(range(N_CORES)))
    except Exception:
        # transient NRT device errors have been observed through the axon
        # tunnel; one retry normally succeeds
        import time as _time
        _time.sleep(2.0)
        res = run_bass_kernel_spmd(nc, in_maps, list(range(N_CORES)))
    out = np.empty((B, T, OUT), np.float32)
    for c in range(N_CORES):
        r = res.results[c]["outT"]                     # [OC, 128, T, BC]
        out[c * BC:(c + 1) * BC] = r.transpose(3, 2, 0, 1).reshape(BC, T, OUT)
    return out

